# revision 56
# baseline (speedup 1.0000x reference)
"""Trainium2 Bass kernel for nn_BatchFlipLoss (NCE batch-flip loss + CE loss).

Math reformulation (validated ~1e-7 vs the jax reference in f64; the
first-order series below adds ~9e-5, vs a 2e-2 gate):

The reference sums BatchCriterion over 36 flip-class pairs (i,j), j>=i.
For pair (i,j) with x = [f_i; f_j] (f_c = features[c::8], L2-normalized,
B=512 rows each), T=0.1, the loss decomposes over ordered halves (a,b).
With E_ab = exp(10*G_ab), G_ab = f_a@f_b.T, S_ab = rowsum(E_ab),
d_ab[p] = f_a[p].f_b[p]:

  D_ab = S0_aa + S_ab      (S0_aa: diag-removed; (a,a): D = 2*S0_aa+e^10)
  half = 10*d - ln(D) - N1/D - ln(1 - exp(10 d)/D),  N1 = S0_aa + S_ab
  (a,a) pair = 2*(10*d - lnD - 2*S0_aa/D)
  ln(1-x) ~ -x only (the x^2/2 tail is ~9e-5 relative after scaling).

Work assignment: 36 unordered blocks over 8 cores = 4.5 each. Core c
computes its diag block (c,c) and blocks (c,c+1), (c,c+2) in full, plus
HALF of its distance-3 and distance-4 blocks: for pair {a, a+k}
(k=3,4), core a takes columns 0:256 of E(f_a rows x f_{a+k} cols) and
core a+k takes rows 256:512 of the mirror block — identical instruction
stream, different host-packed inputs (four [128,256] matmuls per half).
Splitting the late blocks keeps every PSUM colsum chain short so its
staging copy + DMA hides under the final exp groups.

Device pipeline per core (SPMD, inputs rotated so own class is block 0):
  - Gram matmuls write 1-3 tile-slots into multi-bank PSUM tiles; ONE
    ACT exp per group ([128,512..1536]) converts to bf16 E in SBUF.
  - per-slot rowsums: DVE tensor_scalar(+accum_out) on the bf16 E (4x
    DVE mode, accum free) -> out[:, col]; the last group is a single
    diag tile whose rowsum rides the exp's own accumulator, so only ACT
    gates the output DMA.
  - colsums (the partner core's rowsums): PE matmuls with one-hot lhsT
    accumulate j1/j2 chains into a [2,512] PSUM tile (closed mid-kernel)
    and the d3/d4 quarter chains into a [6,256] tile (closed one group
    before the last two cs-free diag groups).
  - CE: one ACT exp over [128,1600] predicts + DVE accum per 400-chunk.
  - diag of block (c,c) is NOT zeroed on device: the host subtracts
    exp(10*||f_p||^2_bf16) from the raw diag rowsums.
The host does only O(N*D)/O(N) work: input layout, d_ab products, the
CE label gather, and the closed-form scalar combine.
"""

from contextlib import ExitStack

import numpy as np

FLIP = 8
B = 512
D = 128
C = 400
N = 4096
ALPHA = 0.03
E10 = float(np.exp(np.float32(10.0)))

# ftp column layout (bf16)
J1 = 512               # 512:1024   distance-1 block
J2 = 1024              # 1024:1536  distance-2 block
R3P0 = 1536            # 1536:2048  d3 P0 rhs (partner[0:256] twice)
R3P1 = 2048            # 2048:2560  d3 P1 rhs (mirror partner, full)
L4 = 2560              # 2560:3072  d4 lhsT chunks (parity-packed)
R4 = 3072              # 3072:3584  d4 rhs halves (parity-packed)
OHJ = 3584             # 3584:3609  one-hots ([128,5] x5) for the cshp rows
OH6 = 3609             # 3609:3645  one-hots for the 6 quarter-chain rows
FT_COLS = 3648

# slot ids: s0 = diag chunk0 | s1..s4 = j1 r0..r3 | s5..s8 = j2 r0..r3 |
# s9,s10 = d3-P0 halves | s11,s12 = d3-P1 halves | s13,s14 = d4 halves |
# s15,s16,s17 = diag chunks 1,2,3.
# The diag block is lower-triangular: chunk r covers cols 0:128(r+1); the
# upper-triangle mass comes back via per-chunk colsums (cshp rows 2-4).
# outt rowsum cols: s0->0, s1..s8 -> 1..8, half-slot quarters -> 9..20,
# diag chunks 1..3 -> 21..23, CE -> 24..27.
GROUP_LAYOUT = [
    [(16, 0, 384)],                                   # A: diag chunk2
    [(17, 0, 512), (15, 512, 256)],                   # B: chunks 3,1 (no junk)
    [(1, 0, 512), (2, 512, 512), (3, 1024, 512)],     # C: j1 r0-2
    [(4, 0, 512), (5, 512, 512), (6, 1024, 512)],     # D: j1r3, j2r0, j2r1
    [(7, 0, 512), (8, 512, 512), (9, 1024, 512)],     # E: j2r2, j2r3, d3p0a
    [(10, 0, 512), (11, 512, 512), (12, 1024, 512)],  # F: d3 halves
    [(13, 0, 512), (14, 512, 512)],                   # G: d4 halves
    [(0, 0, 128)],                                    # H: diag chunk0
]
J0_COL = {0: 0, 1: 21, 2: 22, 3: 23}
TRI_ROW = {1: 2, 2: 3, 3: 4}   # diag chunk -> cshp colsum row
# half-slot kind -> (first quarter's outt col, cs4 row base, lhs offsets, rhs base)
HALF_KIND = {
    "d3p0": (9, 2, (0, 128, 256, 384), R3P0),
    "d3p1": (13, 4, (256, 384, 256, 384), R3P1),
    "d4": (17, 0, (L4, L4 + 128, L4 + 256, L4 + 384), R4),
}

_CACHE = {}


def _slot_info(s):
    if s == 0:
        return ("tri", None, 0)
    if 1 <= s <= 8:
        return ("full", 1 + (s - 1) // 4, (s - 1) % 4)
    if s in (9, 10):
        return ("d3p0", None, s - 9)
    if s in (11, 12):
        return ("d3p1", None, s - 11)
    if s in (13, 14):
        return ("d4", None, s - 13)
    return ("tri", None, s - 14)


def _build_nc():
    import concourse.tile as tile
    from concourse import bacc, mybir

    f32 = mybir.dt.float32
    bf16 = mybir.dt.bfloat16
    AF = mybir.ActivationFunctionType
    OP = mybir.AluOpType

    nc = bacc.Bacc("TRN2", target_bir_lowering=False, debug=False)

    ft_d = nc.dram_tensor("ft", [D, FT_COLS], bf16, kind="ExternalInput")
    pred_d = nc.dram_tensor("pred", [128, 4 * C], f32, kind="ExternalInput")
    out_d = nc.dram_tensor("out", [128, 28], f32, kind="ExternalOutput")
    csj_d = nc.dram_tensor("csj", [5, B], f32, kind="ExternalOutput")
    cs4_d = nc.dram_tensor("cs4", [6, 256], f32, kind="ExternalOutput")

    with tile.TileContext(nc) as tc, ExitStack() as ctx:
        const = ctx.enter_context(tc.tile_pool(name="const", bufs=1))
        gpool = ctx.enter_context(tc.tile_pool(name="gp", bufs=2, space="PSUM"))
        cjpool = ctx.enter_context(tc.tile_pool(name="cj", bufs=1, space="PSUM"))
        c4pool = ctx.enter_context(tc.tile_pool(name="c4", bufs=1, space="PSUM"))
        epool = ctx.enter_context(tc.tile_pool(name="ep", bufs=4))
        small = ctx.enter_context(tc.tile_pool(name="sm", bufs=1))

        ftt = const.tile([D, FT_COLS], bf16)
        predt = const.tile([128, 4 * C], f32)
        outt = small.tile([128, 28], f32)
        scr = small.tile([128, 3 * B], bf16)
        scrp = small.tile([128, C], bf16)
        csjs = small.tile([5, B], f32)
        cs4s = small.tile([6, 256], f32)

        # the one-hot columns are constants — build them with Pool memsets
        # so the early colsum matmuls don't wait for the last ft DMA
        nc.gpsimd.memset(ftt[:, OHJ:FT_COLS], 0.0)
        for k in range(5):
            nc.gpsimd.memset(ftt[:, OHJ + 5 * k + k : OHJ + 5 * k + k + 1], 1.0)
        for r in range(6):
            nc.gpsimd.memset(ftt[:, OH6 + 6 * r + r : OH6 + 6 * r + r + 1], 1.0)
        # group A needs only cols 0:384; splitting here trims ~90ns off the
        # first transfer's latency chain (group structure unchanged)
        nc.sync.dma_start(ftt[:, 0:384], ft_d[:, 0:384])
        nc.sync.dma_start(ftt[:, 384:B], ft_d[:, 384:B])
        nc.sync.dma_start(ftt[:, B : 2 * B], ft_d[:, B : 2 * B])
        nc.sync.dma_start(ftt[:, 2 * B : 3 * B], ft_d[:, 2 * B : 3 * B])
        nc.sync.dma_start(ftt[:, 3 * B : OHJ], ft_d[:, 3 * B : OHJ])
        nc.sync.dma_start(predt[:], pred_d[:, :])

        # one shared bank: j1/j2 colsums live in partitions 0:2 until their
        # staging copy; the final group's Gram tile then overwrites it
        cshp = cjpool.tile([128, B], f32)
        cs4p = c4pool.tile([6, 256], f32)

        pg = {}
        eg = {}

        def emit_mms(g, tile_=None):
            layout = GROUP_LAYOUT[g]
            width = max(off + w for _, off, w in layout)
            if tile_ is None:
                pgt = gpool.tile([128, width], f32, tag="pg")
            else:
                pgt = tile_
            pg[g] = pgt
            for s, off, w in layout:
                kind, j, r = _slot_info(s)
                if kind == "tri":
                    nc.tensor.matmul(
                        pgt[:, off : off + w],
                        ftt[:, r * 128 : (r + 1) * 128],
                        ftt[:, 0:w],
                        start=True,
                        stop=True,
                    )
                elif kind == "full":
                    nc.tensor.matmul(
                        pgt[:, off : off + B],
                        ftt[:, r * 128 : (r + 1) * 128],
                        ftt[:, j * B : (j + 1) * B],
                        start=True,
                        stop=True,
                    )
                else:
                    lhs_off, rhs_base = HALF_KIND[kind][2], HALF_KIND[kind][3]
                    for h in range(2):
                        q = 2 * r + h
                        lo = lhs_off[q] if kind != "d3p0" else 128 * q
                        nc.tensor.matmul(
                            pgt[:, off + h * 256 : off + (h + 1) * 256],
                            ftt[:, lo : lo + 128],
                            ftt[:, rhs_base + (q // 2) * 256 : rhs_base + (q // 2 + 1) * 256],
                            start=True,
                            stop=True,
                        )

        def emit_exp(g, accum=None):
            width = max(off + w for _, off, w in GROUP_LAYOUT[g])
            egt = epool.tile([128, width], bf16, tag="eg")
            eg[g] = egt
            nc.scalar.activation(
                egt[:], pg[g][:, 0:width], AF.Exp, bias=0.0, scale=10.0,
                accum_out=accum,
            )

        def emit_rs(g):
            for s, off, w in GROUP_LAYOUT[g]:
                kind, j, r = _slot_info(s)
                if kind == "tri":
                    col = J0_COL[r]
                    nc.vector.tensor_scalar(
                        scr[:, 0:w],
                        eg[g][:, off : off + w],
                        1.0, None, OP.mult, OP.add,
                        accum_out=outt[:, col : col + 1],
                    )
                elif kind == "full":
                    nc.vector.tensor_scalar(
                        scr[:, off : off + B],
                        eg[g][:, off : off + B],
                        1.0, None, OP.mult, OP.add,
                        accum_out=outt[:, s : s + 1],
                    )
                else:
                    col0 = HALF_KIND[kind][0]
                    for h in range(2):
                        col = col0 + 2 * r + h
                        nc.vector.tensor_scalar(
                            scr[:, off + h * 256 : off + (h + 1) * 256],
                            eg[g][:, off + h * 256 : off + (h + 1) * 256],
                            1.0, None, OP.mult, OP.add,
                            accum_out=outt[:, col : col + 1],
                        )

        def emit_cs(g):
            # cshp chain: row 0/1 = j1/j2 colsums, rows 2-4 = diag-chunk
            # colsums.  start on the (chronologically first) chunk-3 matmul,
            # stop on the full-width j2r3 one.
            for s, off, w in GROUP_LAYOUT[g]:
                kind, j, r = _slot_info(s)
                if kind == "tri":
                    if r == 0:
                        continue
                    oh = OHJ + 5 * TRI_ROW[r]
                    nc.tensor.matmul(
                        cshp[0:5, 0:w],
                        ftt[:, oh : oh + 5],
                        eg[g][:, off : off + w],
                        start=(s == 17),
                        stop=False,
                    )
                elif kind == "full":
                    if j not in (1, 2):
                        continue
                    oh = OHJ + 5 * (j - 1)
                    nc.tensor.matmul(
                        cshp[0:5, :],
                        ftt[:, oh : oh + 5],
                        eg[g][:, off : off + B],
                        start=False,
                        stop=(s == 8),
                    )
                else:
                    row_base = HALF_KIND[kind][1]
                    for h in range(2):
                        q = 2 * r + h
                        row = row_base + q // 2
                        oh = OH6 + 6 * row
                        nc.tensor.matmul(
                            cs4p[:],
                            ftt[:, oh : oh + 6],
                            eg[g][:, off + h * 256 : off + (h + 1) * 256],
                            start=(s == 9 and q == 0),
                            stop=(s == 14 and q == 3),
                        )

        # Explicit pipeline schedule.  ACT order: exps 0-6, CE, 7 — the CE
        # exp covers the closing colsum chains' staging copies + Pool-queue
        # DMAs, and the last group is the tiny diag chunk-0 tile whose
        # rowsum rides the exp accumulator.
        # group A borrows the colsum bank (free until the chain's first
        # matmul, which runs after exp A) so group C gets a fresh rotation
        # slot and isn't WAR-blocked behind exp A
        emit_mms(0, tile_=cshp)
        emit_mms(1)
        emit_exp(0)
        emit_rs(0)
        emit_mms(2)
        emit_exp(1)
        emit_rs(1)
        emit_mms(3)
        emit_exp(2)
        emit_rs(2)
        emit_cs(1)       # diag chunks 3,1 colsums (t3 opens the chain)
        emit_cs(0)       # diag chunk 2 colsum, after the opener
        emit_mms(4)
        emit_exp(3)
        emit_rs(3)
        emit_cs(2)       # j1 r0-2
        emit_mms(5)
        emit_exp(4)
        emit_rs(4)
        emit_cs(3)       # j1r3, j2r0, j2r1
        emit_mms(6)
        emit_exp(5)
        # cs(4) closes the cshp chain (j2r3 is the stop); stage + DMA in
        # DVE slack, then the final group's matmul recycles the bank
        emit_cs(4)
        nc.vector.tensor_copy(csjs[:], cshp[0:5, :])
        nc.gpsimd.dma_start(csj_d[:, :], csjs[:])
        emit_rs(5)
        emit_mms(7, tile_=cshp)
        emit_exp(6)
        emit_rs(6)
        emit_cs(5)       # d3 half quarters
        emit_cs(6)       # d4 quarters -> cs4p stop
        nc.vector.tensor_copy(cs4s[:], cs4p[:])
        nc.gpsimd.dma_start(cs4_d[:, :], cs4s[:])
        # CE: chunks 0-2 as one exp with DVE accums (they finish under the
        # final exps); chunk 3 as its own exp riding the ACT accumulator so
        # no DVE work gates the output DMA
        ept = epool.tile([128, 3 * C], bf16, tag="ept")
        nc.scalar.activation(ept[:], predt[:, 0 : 3 * C], AF.Exp, bias=0.0, scale=1.0)
        for cchunk in range(3):
            nc.vector.tensor_scalar(
                scrp[:],
                ept[:, cchunk * C : (cchunk + 1) * C],
                1.0, None, OP.mult, OP.add,
                accum_out=outt[:, 24 + cchunk : 25 + cchunk],
            )
        ept2 = epool.tile([128, C], bf16, tag="ept2")
        nc.scalar.activation(
            ept2[:], predt[:, 3 * C : 4 * C], AF.Exp, bias=0.0, scale=1.0,
            accum_out=outt[:, 27:28],
        )
        # last group: diag chunk 0, rowsum via the exp's accumulator
        # its exp output is never read (only the accumulator is), so write
        # it into spare PSUM columns: both operands PSUM -> cheaper access
        nc.scalar.activation(
            cshp[:, 128:256], cshp[:, 0:128], AF.Exp, bias=0.0, scale=10.0,
            accum_out=outt[:, 0:1],
        )

        nc.sync.dma_start(out_d[:, :], outt[:])

    nc.compile()
    return nc


def _get_nc():
    if "nc" not in _CACHE:
        _CACHE["nc"] = _build_nc()
    return _CACHE["nc"]


def _prep_in_maps(predicts, labels, features):
    import ml_dtypes

    feats = np.ascontiguousarray(features, dtype=np.float32)
    pred = np.ascontiguousarray(predicts, dtype=np.float32)
    f8 = feats.reshape(B, FLIP, D).transpose(1, 0, 2)  # [8,512,128], f8[c]=feats[c::8]

    # cshp one-hots: rows 0/1 = j1/j2, rows 2-4 = diag chunks 1-3
    ohj = np.zeros((D, 25), dtype=np.float32)
    for k in range(5):
        ohj[:, 5 * k + k] = 1.0
    oh6 = np.zeros((D, 36), dtype=np.float32)
    for r in range(6):
        oh6[:, 6 * r + r] = 1.0

    in_maps = []
    for a in range(FLIP):
        ft = np.zeros((D, FT_COLS), dtype=np.float32)
        ft[:, 0:B] = f8[a].T
        ft[:, J1 : J1 + B] = f8[(a + 1) % FLIP].T
        ft[:, J2 : J2 + B] = f8[(a + 2) % FLIP].T
        p3 = f8[(a + 3) % FLIP]
        ft[:, R3P0 : R3P0 + 256] = p3[0:256].T
        ft[:, R3P0 + 256 : R3P1] = p3[0:256].T
        ft[:, R3P1 : R3P1 + B] = f8[(a - 3) % FLIP].T
        p4 = f8[(a + 4) % FLIP]
        own = f8[a]
        if a < 4:
            ft[:, L4 : L4 + B] = own.T
            ft[:, R4 : R4 + 256] = p4[0:256].T
            ft[:, R4 + 256 : R4 + 512] = p4[0:256].T
        else:
            ft[:, L4 : L4 + 128] = own[256:384].T
            ft[:, L4 + 128 : L4 + 256] = own[384:512].T
            ft[:, L4 + 256 : L4 + 384] = own[256:384].T
            ft[:, L4 + 384 : R4] = own[384:512].T
            ft[:, R4 : R4 + B] = p4.T
        ft[:, OHJ:OH6] = ohj
        ft[:, OH6 : OH6 + 36] = oh6
        pb = pred[a * B : (a + 1) * B].reshape(4, 128, C).transpose(1, 0, 2)
        in_maps.append(
            {
                "ft": np.ascontiguousarray(ft).astype(ml_dtypes.bfloat16),
                "pred": np.ascontiguousarray(pb.reshape(128, 4 * C)),
            }
        )
    return in_maps


def _stitch_pair(mP0, mP1, csP0, csP1, colP0, colP1, rowP0, rowP1):
    """Assemble both rowsum directions of a split block M (P0 core holds
    cols 0:256 over all rows; P1 core holds rows 256:512 over all cols)."""
    partial = mP0[:, colP0 : colP0 + 4].T.reshape(B)      # cols 0:256, by chunk
    compl_ = np.concatenate([csP1[rowP1], csP1[rowP1 + 1]])  # cols 256:512
    s_fwd = partial + compl_
    lo = csP0[rowP0] + csP0[rowP0 + 1]                    # mirror rows 0:256
    hi = np.empty(256)
    hi[0:128] = mP1[:, colP1] + mP1[:, colP1 + 2]         # rows 256:384
    hi[128:256] = mP1[:, colP1 + 1] + mP1[:, colP1 + 3]   # rows 384:512
    s_rev = np.concatenate([lo, hi])
    return s_fwd, s_rev


def _combine(outs, predicts, labels, features):
    """Host-side O(N*D) combine: reroute per-block sums between the
    ordered halves and apply the closed-form first-order series."""
    import ml_dtypes

    feats = np.asarray(features, dtype=np.float32)
    f8 = feats.reshape(B, FLIP, D).transpose(1, 0, 2).astype(np.float64)
    fb8 = f8.astype(ml_dtypes.bfloat16).astype(np.float64)  # device-side values

    dv = np.einsum("apd,bpd->abp", f8, f8)

    m = {}
    csj = {}
    cs4 = {}
    for c in range(FLIP):
        m[c] = np.asarray(outs[c]["out"], np.float64)
        csj[c] = np.asarray(outs[c]["csj"], np.float64)
        cs4[c] = np.asarray(outs[c]["cs4"], np.float64)

    S1 = {}
    for c in range(FLIP):
        # diag: lower-triangle direct sums + the per-chunk colsum rows
        # that carry the upper-triangle mass back
        diag = m[c][:, [J0_COL[r] for r in range(4)]].T.reshape(B).copy()
        diag[0:128] += csj[c][2][0:128]
        diag[0:256] += csj[c][3][0:256]
        diag[0:384] += csj[c][4][0:384]
        S1[(c, c)] = diag
        for j in (1, 2):
            cols = [1 + 4 * (j - 1) + r for r in range(4)]
            S1[(c, (c + j) % FLIP)] = m[c][:, cols].T.reshape(B)
            S1[((c + j) % FLIP, c)] = csj[c][j - 1]

    for b in range(FLIP):  # distance-3 pairs, P0 = core b, P1 = core b+3
        bp = (b + 3) % FLIP
        s_fwd, s_rev = _stitch_pair(m[b], m[bp], cs4[b], cs4[bp], 9, 13, 2, 4)
        S1[(b, bp)] = s_fwd
        S1[(bp, b)] = s_rev
    for b in range(4):     # distance-4 pairs, P0 = core b, P1 = core b+4
        bp = b + 4
        s_fwd, s_rev = _stitch_pair(m[b], m[bp], cs4[b], cs4[bp], 17, 17, 0, 0)
        S1[(b, bp)] = s_fwd
        S1[(bp, b)] = s_rev

    # remove the raw diagonal exp from the own-block rowsums.  chunks 1-3
    # were summed from bf16 E by DVE; chunk 0 rides the ACT accumulator
    # (f32 activation results), so skip the bf16 rounding there.
    S10 = {}
    for c in range(FLIP):
        gpp = np.einsum("pd,pd->p", fb8[c], fb8[c])
        dg = np.exp(10.0 * gpp).astype(np.float32)
        dgb = dg.astype(ml_dtypes.bfloat16).astype(np.float64)
        dgb[0:128] = dg[0:128]
        S10[c] = S1[(c, c)] - dgb

    nce = 0.0
    for a in range(FLIP):
        for b in range(FLIP):
            d = dv[a, b]
            if a == b:
                N1 = 2.0 * S10[a]
                Dv = N1 + E10
                half = 10.0 * d - np.log(Dv) - N1 / Dv
                nce += 2.0 * half.sum()
            else:
                N1 = S10[a] + S1[(a, b)]
                half = (
                    10.0 * d
                    - np.log(N1)
                    - 1.0
                    - np.log1p(-np.exp(10.0 * d) / N1)
                )
                nce += half.sum()

    # CE: device exp-sums + host label gather
    pred = np.asarray(predicts, dtype=np.float64)
    lab = np.asarray(labels).astype(np.int64)
    xl = pred[np.arange(N), lab]
    ce = -xl.sum()
    for c in range(FLIP):
        se = m[c][:, 24:28]  # se[p, cc] = sum_k exp(pred[c*512+cc*128+p, k])
        ce += np.log(se).T.reshape(B).sum()

    val = ALPHA * (-(nce) / 1024.0) + ce / N
    return np.array(val, dtype=np.float32)


def _run_hw(in_maps, trace=False):
    from concourse.bass_utils import run_bass_kernel_spmd

    nc = _get_nc()
    return run_bass_kernel_spmd(nc, in_maps, core_ids=list(range(FLIP)), trace=trace)


def kernel(predicts, labels, features, indexs=None, **_):
    in_maps = _prep_in_maps(predicts, labels, features)
    res = _run_hw(in_maps)
    return _combine(res.results, predicts, labels, features)


def kernel_sim(predicts, labels, features, indexs=None, **_):
    """CoreSim (CPU simulator) path for fast correctness iteration."""
    from concourse.bass_interp import CoreSim

    nc = _get_nc()
    in_maps = _prep_in_maps(predicts, labels, features)
    outs = []
    for a in range(FLIP):
        sim = CoreSim(nc, trace=False)
        for k, v in in_maps[a].items():
            sim.tensor(k)[:] = v
        sim.simulate()
        outs.append({k: np.array(sim.tensor(k)) for k in ("out", "csj", "cs4")})
    return _combine(outs, predicts, labels, features)


# revision 57
# speedup vs baseline: 1.0113x; 1.0113x over previous
"""Trainium2 Bass kernel for nn_BatchFlipLoss (NCE batch-flip loss + CE loss).

Math reformulation (validated ~1e-7 vs the jax reference in f64; the
first-order series below adds ~9e-5, vs a 2e-2 gate):

The reference sums BatchCriterion over 36 flip-class pairs (i,j), j>=i.
For pair (i,j) with x = [f_i; f_j] (f_c = features[c::8], L2-normalized,
B=512 rows each), T=0.1, the loss decomposes over ordered halves (a,b).
With E_ab = exp(10*G_ab), G_ab = f_a@f_b.T, S_ab = rowsum(E_ab),
d_ab[p] = f_a[p].f_b[p]:

  D_ab = S0_aa + S_ab      (S0_aa: diag-removed; (a,a): D = 2*S0_aa+e^10)
  half = 10*d - ln(D) - N1/D - ln(1 - exp(10 d)/D),  N1 = S0_aa + S_ab
  (a,a) pair = 2*(10*d - lnD - 2*S0_aa/D)
  ln(1-x) ~ -x only (the x^2/2 tail is ~9e-5 relative after scaling).

Work assignment: 36 unordered blocks over 8 cores = 4.5 each. Core c
computes its diag block (c,c) and blocks (c,c+1), (c,c+2) in full, plus
HALF of its distance-3 and distance-4 blocks: for pair {a, a+k}
(k=3,4), core a takes columns 0:256 of E(f_a rows x f_{a+k} cols) and
core a+k takes rows 256:512 of the mirror block — identical instruction
stream, different host-packed inputs (four [128,256] matmuls per half).
Splitting the late blocks keeps every PSUM colsum chain short so its
staging copy + DMA hides under the final exp groups.

Device pipeline per core (SPMD, inputs rotated so own class is block 0):
  - Gram matmuls write 1-3 tile-slots into multi-bank PSUM tiles; ONE
    ACT exp per group ([128,512..1536]) converts to bf16 E in SBUF.
  - per-slot rowsums: DVE tensor_scalar(+accum_out) on the bf16 E (4x
    DVE mode, accum free) -> out[:, col]; the last group is a single
    diag tile whose rowsum rides the exp's own accumulator, so only ACT
    gates the output DMA.
  - colsums (the partner core's rowsums): PE matmuls with one-hot lhsT
    accumulate j1/j2 chains into a [2,512] PSUM tile (closed mid-kernel)
    and the d3/d4 quarter chains into a [6,256] tile (closed one group
    before the last two cs-free diag groups).
  - CE: one ACT exp over [128,1600] predicts + DVE accum per 400-chunk.
  - diag of block (c,c) is NOT zeroed on device: the host subtracts
    exp(10*||f_p||^2_bf16) from the raw diag rowsums.
The host does only O(N*D)/O(N) work: input layout, d_ab products, the
CE label gather, and the closed-form scalar combine.
"""

from contextlib import ExitStack

import numpy as np

FLIP = 8
B = 512
D = 128
C = 400
N = 4096
ALPHA = 0.03
E10 = float(np.exp(np.float32(10.0)))

# ftp column layout (bf16)
J1 = 512               # 512:1024   distance-1 block
J2 = 1024              # 1024:1536  distance-2 block
R3P0 = 1536            # 1536:2048  d3 P0 rhs (partner[0:256] twice)
R3P1 = 2048            # 2048:2560  d3 P1 rhs (mirror partner, full)
L4 = 2560              # 2560:3072  d4 lhsT chunks (parity-packed)
R4 = 3072              # 3072:3584  d4 rhs halves (parity-packed)
OHJ = 3584             # 3584:3609  one-hots ([128,5] x5) for the cshp rows
OH6 = 3609             # 3609:3645  one-hots for the 6 quarter-chain rows
FT_COLS = 3648

# slot ids: s0 = diag chunk0 | s1..s4 = j1 r0..r3 | s5..s8 = j2 r0..r3 |
# s9,s10 = d3-P0 halves | s11,s12 = d3-P1 halves | s13,s14 = d4 halves |
# s15,s16,s17 = diag chunks 1,2,3.
# The diag block is lower-triangular: chunk r covers cols 0:128(r+1); the
# upper-triangle mass comes back via per-chunk colsums (cshp rows 2-4).
# outt rowsum cols: s0->0, s1..s8 -> 1..8, half-slot quarters -> 9..20,
# diag chunks 1..3 -> 21..23, CE -> 24..27.
GROUP_LAYOUT = [
    [(16, 0, 384)],                                   # A: diag chunk2
    [(17, 0, 512), (15, 512, 256)],                   # B: chunks 3,1 (no junk)
    [(1, 0, 512), (2, 512, 512), (3, 1024, 512)],     # C: j1 r0-2
    [(4, 0, 512), (5, 512, 512), (6, 1024, 512)],     # D: j1r3, j2r0, j2r1
    [(7, 0, 512), (8, 512, 512), (9, 1024, 512)],     # E: j2r2, j2r3, d3p0a
    [(10, 0, 512), (11, 512, 512), (12, 1024, 512)],  # F: d3 halves
    [(13, 0, 512), (14, 512, 512)],                   # G: d4 halves
    [(0, 0, 128)],                                    # H: diag chunk0
]
J0_COL = {0: 0, 1: 21, 2: 22, 3: 23}
TRI_ROW = {1: 2, 2: 3, 3: 4}   # diag chunk -> cshp colsum row
# half-slot kind -> (first quarter's outt col, cs4 row base, lhs offsets, rhs base)
HALF_KIND = {
    "d3p0": (9, 2, (0, 128, 256, 384), R3P0),
    "d3p1": (13, 4, (256, 384, 256, 384), R3P1),
    "d4": (17, 0, (L4, L4 + 128, L4 + 256, L4 + 384), R4),
}

_CACHE = {}


def _slot_info(s):
    if s == 0:
        return ("tri", None, 0)
    if 1 <= s <= 8:
        return ("full", 1 + (s - 1) // 4, (s - 1) % 4)
    if s in (9, 10):
        return ("d3p0", None, s - 9)
    if s in (11, 12):
        return ("d3p1", None, s - 11)
    if s in (13, 14):
        return ("d4", None, s - 13)
    return ("tri", None, s - 14)


def _build_nc():
    import concourse.tile as tile
    from concourse import bacc, mybir

    f32 = mybir.dt.float32
    bf16 = mybir.dt.bfloat16
    AF = mybir.ActivationFunctionType
    OP = mybir.AluOpType

    nc = bacc.Bacc("TRN2", target_bir_lowering=False, debug=False)

    ft_d = nc.dram_tensor("ft", [D, FT_COLS], bf16, kind="ExternalInput")
    pred_d = nc.dram_tensor("pred", [128, 4 * C], f32, kind="ExternalInput")
    out_d = nc.dram_tensor("out", [128, 28], f32, kind="ExternalOutput")
    csj_d = nc.dram_tensor("csj", [5, B], f32, kind="ExternalOutput")
    cs4_d = nc.dram_tensor("cs4", [6, 256], f32, kind="ExternalOutput")

    with tile.TileContext(nc) as tc, ExitStack() as ctx:
        const = ctx.enter_context(tc.tile_pool(name="const", bufs=1))
        gpool = ctx.enter_context(tc.tile_pool(name="gp", bufs=2, space="PSUM"))
        cjpool = ctx.enter_context(tc.tile_pool(name="cj", bufs=1, space="PSUM"))
        c4pool = ctx.enter_context(tc.tile_pool(name="c4", bufs=1, space="PSUM"))
        epool = ctx.enter_context(tc.tile_pool(name="ep", bufs=4))
        small = ctx.enter_context(tc.tile_pool(name="sm", bufs=1))

        ftt = const.tile([D, FT_COLS], bf16)
        predt = const.tile([128, 4 * C], f32)
        outt = small.tile([128, 28], f32)
        scr = small.tile([128, 3 * B], bf16)
        scrp = small.tile([128, C], bf16)
        csjs = small.tile([5, B], f32)
        cs4s = small.tile([6, 256], f32)

        # the one-hot columns are constants — build them with Pool memsets
        # so the early colsum matmuls don't wait for the last ft DMA
        nc.gpsimd.memset(ftt[:, OHJ:FT_COLS], 0.0)
        for k in range(5):
            nc.gpsimd.memset(ftt[:, OHJ + 5 * k + k : OHJ + 5 * k + k + 1], 1.0)
        for r in range(6):
            nc.gpsimd.memset(ftt[:, OH6 + 6 * r + r : OH6 + 6 * r + r + 1], 1.0)
        nc.sync.dma_start(ftt[:, 0:B], ft_d[:, 0:B])
        nc.sync.dma_start(ftt[:, B : 2 * B], ft_d[:, B : 2 * B])
        nc.sync.dma_start(ftt[:, 2 * B : 3 * B], ft_d[:, 2 * B : 3 * B])
        nc.sync.dma_start(ftt[:, 3 * B : OHJ], ft_d[:, 3 * B : OHJ])
        nc.sync.dma_start(predt[:], pred_d[:, :])

        # one shared bank: j1/j2 colsums live in partitions 0:2 until their
        # staging copy; the final group's Gram tile then overwrites it
        cshp = cjpool.tile([128, B], f32)
        cs4p = c4pool.tile([6, 256], f32)

        pg = {}
        eg = {}

        def emit_mms(g, tile_=None):
            layout = GROUP_LAYOUT[g]
            width = max(off + w for _, off, w in layout)
            if tile_ is None:
                pgt = gpool.tile([128, width], f32, tag="pg")
            else:
                pgt = tile_
            pg[g] = pgt
            for s, off, w in layout:
                kind, j, r = _slot_info(s)
                if kind == "tri":
                    nc.tensor.matmul(
                        pgt[:, off : off + w],
                        ftt[:, r * 128 : (r + 1) * 128],
                        ftt[:, 0:w],
                        start=True,
                        stop=True,
                    )
                elif kind == "full":
                    nc.tensor.matmul(
                        pgt[:, off : off + B],
                        ftt[:, r * 128 : (r + 1) * 128],
                        ftt[:, j * B : (j + 1) * B],
                        start=True,
                        stop=True,
                    )
                else:
                    lhs_off, rhs_base = HALF_KIND[kind][2], HALF_KIND[kind][3]
                    for h in range(2):
                        q = 2 * r + h
                        lo = lhs_off[q] if kind != "d3p0" else 128 * q
                        nc.tensor.matmul(
                            pgt[:, off + h * 256 : off + (h + 1) * 256],
                            ftt[:, lo : lo + 128],
                            ftt[:, rhs_base + (q // 2) * 256 : rhs_base + (q // 2 + 1) * 256],
                            start=True,
                            stop=True,
                        )

        def emit_exp(g, accum=None):
            width = max(off + w for _, off, w in GROUP_LAYOUT[g])
            egt = epool.tile([128, width], bf16, tag="eg")
            eg[g] = egt
            nc.scalar.activation(
                egt[:], pg[g][:, 0:width], AF.Exp, bias=0.0, scale=10.0,
                accum_out=accum,
            )

        def emit_rs(g):
            for s, off, w in GROUP_LAYOUT[g]:
                kind, j, r = _slot_info(s)
                if kind == "tri":
                    col = J0_COL[r]
                    nc.vector.tensor_scalar(
                        scr[:, 0:w],
                        eg[g][:, off : off + w],
                        1.0, None, OP.mult, OP.add,
                        accum_out=outt[:, col : col + 1],
                    )
                elif kind == "full":
                    nc.vector.tensor_scalar(
                        scr[:, off : off + B],
                        eg[g][:, off : off + B],
                        1.0, None, OP.mult, OP.add,
                        accum_out=outt[:, s : s + 1],
                    )
                else:
                    col0 = HALF_KIND[kind][0]
                    for h in range(2):
                        col = col0 + 2 * r + h
                        nc.vector.tensor_scalar(
                            scr[:, off + h * 256 : off + (h + 1) * 256],
                            eg[g][:, off + h * 256 : off + (h + 1) * 256],
                            1.0, None, OP.mult, OP.add,
                            accum_out=outt[:, col : col + 1],
                        )

        def emit_cs(g):
            # cshp chain: row 0/1 = j1/j2 colsums, rows 2-4 = diag-chunk
            # colsums.  start on the (chronologically first) chunk-3 matmul,
            # stop on the full-width j2r3 one.
            for s, off, w in GROUP_LAYOUT[g]:
                kind, j, r = _slot_info(s)
                if kind == "tri":
                    if r == 0:
                        continue
                    oh = OHJ + 5 * TRI_ROW[r]
                    nc.tensor.matmul(
                        cshp[0:5, 0:w],
                        ftt[:, oh : oh + 5],
                        eg[g][:, off : off + w],
                        start=(s == 17),
                        stop=False,
                    )
                elif kind == "full":
                    if j not in (1, 2):
                        continue
                    oh = OHJ + 5 * (j - 1)
                    nc.tensor.matmul(
                        cshp[0:5, :],
                        ftt[:, oh : oh + 5],
                        eg[g][:, off : off + B],
                        start=False,
                        stop=(s == 8),
                    )
                else:
                    row_base = HALF_KIND[kind][1]
                    for h in range(2):
                        q = 2 * r + h
                        row = row_base + q // 2
                        oh = OH6 + 6 * row
                        nc.tensor.matmul(
                            cs4p[:],
                            ftt[:, oh : oh + 6],
                            eg[g][:, off + h * 256 : off + (h + 1) * 256],
                            start=(s == 9 and q == 0),
                            stop=(s == 14 and q == 3),
                        )

        # Explicit pipeline schedule.  ACT order: exps 0-6, CE, 7 — the CE
        # exp covers the closing colsum chains' staging copies + Pool-queue
        # DMAs, and the last group is the tiny diag chunk-0 tile whose
        # rowsum rides the exp accumulator.
        # group A borrows the colsum bank (free until the chain's first
        # matmul, which runs after exp A) so group C gets a fresh rotation
        # slot and isn't WAR-blocked behind exp A
        emit_mms(0, tile_=cshp)
        emit_mms(1)
        emit_exp(0)
        emit_rs(0)
        emit_mms(2)
        emit_exp(1)
        emit_rs(1)
        emit_mms(3)
        emit_exp(2)
        emit_rs(2)
        emit_cs(1)       # diag chunks 3,1 colsums (t3 opens the chain)
        emit_cs(0)       # diag chunk 2 colsum, after the opener
        emit_mms(4)
        emit_exp(3)
        emit_rs(3)
        emit_cs(2)       # j1 r0-2
        emit_mms(5)
        emit_exp(4)
        emit_rs(4)
        emit_cs(3)       # j1r3, j2r0, j2r1
        emit_mms(6)
        emit_exp(5)
        # cs(4) closes the cshp chain (j2r3 is the stop); stage + DMA in
        # DVE slack, then the final group's matmul recycles the bank
        emit_cs(4)
        nc.vector.tensor_copy(csjs[:], cshp[0:5, :])
        nc.gpsimd.dma_start(csj_d[:, :], csjs[:])
        emit_rs(5)
        emit_mms(7, tile_=cshp)
        emit_exp(6)
        emit_rs(6)
        emit_cs(5)       # d3 half quarters
        emit_cs(6)       # d4 quarters -> cs4p stop
        nc.vector.tensor_copy(cs4s[:], cs4p[:])
        nc.gpsimd.dma_start(cs4_d[:, :], cs4s[:])
        # CE: chunks 0-2 as one exp with DVE accums (they finish under the
        # final exps); chunk 3 as its own exp riding the ACT accumulator so
        # no DVE work gates the output DMA
        ept = epool.tile([128, 3 * C], bf16, tag="ept")
        nc.scalar.activation(ept[:], predt[:, 0 : 3 * C], AF.Exp, bias=0.0, scale=1.0)
        for cchunk in range(3):
            nc.vector.tensor_scalar(
                scrp[:],
                ept[:, cchunk * C : (cchunk + 1) * C],
                1.0, None, OP.mult, OP.add,
                accum_out=outt[:, 24 + cchunk : 25 + cchunk],
            )
        ept2 = epool.tile([128, C], bf16, tag="ept2")
        nc.scalar.activation(
            ept2[:], predt[:, 3 * C : 4 * C], AF.Exp, bias=0.0, scale=1.0,
            accum_out=outt[:, 27:28],
        )
        # last group: diag chunk 0, rowsum via the exp's accumulator
        # its exp output is never read (only the accumulator is), so write
        # it into spare PSUM columns: both operands PSUM -> cheaper access
        nc.scalar.activation(
            cshp[:, 128:256], cshp[:, 0:128], AF.Exp, bias=0.0, scale=10.0,
            accum_out=outt[:, 0:1],
        )

        nc.sync.dma_start(out_d[:, :], outt[:])

    nc.compile()
    return nc


def _get_nc():
    if "nc" not in _CACHE:
        _CACHE["nc"] = _build_nc()
    return _CACHE["nc"]


def _prep_in_maps(predicts, labels, features):
    import ml_dtypes

    feats = np.ascontiguousarray(features, dtype=np.float32)
    pred = np.ascontiguousarray(predicts, dtype=np.float32)
    f8 = feats.reshape(B, FLIP, D).transpose(1, 0, 2)  # [8,512,128], f8[c]=feats[c::8]

    # cshp one-hots: rows 0/1 = j1/j2, rows 2-4 = diag chunks 1-3
    ohj = np.zeros((D, 25), dtype=np.float32)
    for k in range(5):
        ohj[:, 5 * k + k] = 1.0
    oh6 = np.zeros((D, 36), dtype=np.float32)
    for r in range(6):
        oh6[:, 6 * r + r] = 1.0

    in_maps = []
    for a in range(FLIP):
        ft = np.zeros((D, FT_COLS), dtype=np.float32)
        ft[:, 0:B] = f8[a].T
        ft[:, J1 : J1 + B] = f8[(a + 1) % FLIP].T
        ft[:, J2 : J2 + B] = f8[(a + 2) % FLIP].T
        p3 = f8[(a + 3) % FLIP]
        ft[:, R3P0 : R3P0 + 256] = p3[0:256].T
        ft[:, R3P0 + 256 : R3P1] = p3[0:256].T
        ft[:, R3P1 : R3P1 + B] = f8[(a - 3) % FLIP].T
        p4 = f8[(a + 4) % FLIP]
        own = f8[a]
        if a < 4:
            ft[:, L4 : L4 + B] = own.T
            ft[:, R4 : R4 + 256] = p4[0:256].T
            ft[:, R4 + 256 : R4 + 512] = p4[0:256].T
        else:
            ft[:, L4 : L4 + 128] = own[256:384].T
            ft[:, L4 + 128 : L4 + 256] = own[384:512].T
            ft[:, L4 + 256 : L4 + 384] = own[256:384].T
            ft[:, L4 + 384 : R4] = own[384:512].T
            ft[:, R4 : R4 + B] = p4.T
        ft[:, OHJ:OH6] = ohj
        ft[:, OH6 : OH6 + 36] = oh6
        pb = pred[a * B : (a + 1) * B].reshape(4, 128, C).transpose(1, 0, 2)
        in_maps.append(
            {
                "ft": np.ascontiguousarray(ft).astype(ml_dtypes.bfloat16),
                "pred": np.ascontiguousarray(pb.reshape(128, 4 * C)),
            }
        )
    return in_maps


def _stitch_pair(mP0, mP1, csP0, csP1, colP0, colP1, rowP0, rowP1):
    """Assemble both rowsum directions of a split block M (P0 core holds
    cols 0:256 over all rows; P1 core holds rows 256:512 over all cols)."""
    partial = mP0[:, colP0 : colP0 + 4].T.reshape(B)      # cols 0:256, by chunk
    compl_ = np.concatenate([csP1[rowP1], csP1[rowP1 + 1]])  # cols 256:512
    s_fwd = partial + compl_
    lo = csP0[rowP0] + csP0[rowP0 + 1]                    # mirror rows 0:256
    hi = np.empty(256)
    hi[0:128] = mP1[:, colP1] + mP1[:, colP1 + 2]         # rows 256:384
    hi[128:256] = mP1[:, colP1 + 1] + mP1[:, colP1 + 3]   # rows 384:512
    s_rev = np.concatenate([lo, hi])
    return s_fwd, s_rev


def _combine(outs, predicts, labels, features):
    """Host-side O(N*D) combine: reroute per-block sums between the
    ordered halves and apply the closed-form first-order series."""
    import ml_dtypes

    feats = np.asarray(features, dtype=np.float32)
    f8 = feats.reshape(B, FLIP, D).transpose(1, 0, 2).astype(np.float64)
    fb8 = f8.astype(ml_dtypes.bfloat16).astype(np.float64)  # device-side values

    dv = np.einsum("apd,bpd->abp", f8, f8)

    m = {}
    csj = {}
    cs4 = {}
    for c in range(FLIP):
        m[c] = np.asarray(outs[c]["out"], np.float64)
        csj[c] = np.asarray(outs[c]["csj"], np.float64)
        cs4[c] = np.asarray(outs[c]["cs4"], np.float64)

    S1 = {}
    for c in range(FLIP):
        # diag: lower-triangle direct sums + the per-chunk colsum rows
        # that carry the upper-triangle mass back
        diag = m[c][:, [J0_COL[r] for r in range(4)]].T.reshape(B).copy()
        diag[0:128] += csj[c][2][0:128]
        diag[0:256] += csj[c][3][0:256]
        diag[0:384] += csj[c][4][0:384]
        S1[(c, c)] = diag
        for j in (1, 2):
            cols = [1 + 4 * (j - 1) + r for r in range(4)]
            S1[(c, (c + j) % FLIP)] = m[c][:, cols].T.reshape(B)
            S1[((c + j) % FLIP, c)] = csj[c][j - 1]

    for b in range(FLIP):  # distance-3 pairs, P0 = core b, P1 = core b+3
        bp = (b + 3) % FLIP
        s_fwd, s_rev = _stitch_pair(m[b], m[bp], cs4[b], cs4[bp], 9, 13, 2, 4)
        S1[(b, bp)] = s_fwd
        S1[(bp, b)] = s_rev
    for b in range(4):     # distance-4 pairs, P0 = core b, P1 = core b+4
        bp = b + 4
        s_fwd, s_rev = _stitch_pair(m[b], m[bp], cs4[b], cs4[bp], 17, 17, 0, 0)
        S1[(b, bp)] = s_fwd
        S1[(bp, b)] = s_rev

    # remove the raw diagonal exp from the own-block rowsums.  chunks 1-3
    # were summed from bf16 E by DVE; chunk 0 rides the ACT accumulator
    # (f32 activation results), so skip the bf16 rounding there.
    S10 = {}
    for c in range(FLIP):
        gpp = np.einsum("pd,pd->p", fb8[c], fb8[c])
        dg = np.exp(10.0 * gpp).astype(np.float32)
        dgb = dg.astype(ml_dtypes.bfloat16).astype(np.float64)
        dgb[0:128] = dg[0:128]
        S10[c] = S1[(c, c)] - dgb

    nce = 0.0
    for a in range(FLIP):
        for b in range(FLIP):
            d = dv[a, b]
            if a == b:
                N1 = 2.0 * S10[a]
                Dv = N1 + E10
                half = 10.0 * d - np.log(Dv) - N1 / Dv
                nce += 2.0 * half.sum()
            else:
                N1 = S10[a] + S1[(a, b)]
                half = (
                    10.0 * d
                    - np.log(N1)
                    - 1.0
                    - np.log1p(-np.exp(10.0 * d) / N1)
                )
                nce += half.sum()

    # CE: device exp-sums + host label gather
    pred = np.asarray(predicts, dtype=np.float64)
    lab = np.asarray(labels).astype(np.int64)
    xl = pred[np.arange(N), lab]
    ce = -xl.sum()
    for c in range(FLIP):
        se = m[c][:, 24:28]  # se[p, cc] = sum_k exp(pred[c*512+cc*128+p, k])
        ce += np.log(se).T.reshape(B).sum()

    val = ALPHA * (-(nce) / 1024.0) + ce / N
    return np.array(val, dtype=np.float32)


def _run_hw(in_maps, trace=False):
    from concourse.bass_utils import run_bass_kernel_spmd

    nc = _get_nc()
    return run_bass_kernel_spmd(nc, in_maps, core_ids=list(range(FLIP)), trace=trace)


def kernel(predicts, labels, features, indexs=None, **_):
    in_maps = _prep_in_maps(predicts, labels, features)
    res = _run_hw(in_maps)
    return _combine(res.results, predicts, labels, features)


def kernel_sim(predicts, labels, features, indexs=None, **_):
    """CoreSim (CPU simulator) path for fast correctness iteration."""
    from concourse.bass_interp import CoreSim

    nc = _get_nc()
    in_maps = _prep_in_maps(predicts, labels, features)
    outs = []
    for a in range(FLIP):
        sim = CoreSim(nc, trace=False)
        for k, v in in_maps[a].items():
            sim.tensor(k)[:] = v
        sim.simulate()
        outs.append({k: np.array(sim.tensor(k)) for k in ("out", "csj", "cs4")})
    return _combine(outs, predicts, labels, features)
